# revision 1
# baseline (speedup 1.0000x reference)
"""Trainium2 Bass kernel for nn_Attention_28948079575569 (softmax pooling).

Computation (reference):
    u      = tanh(h @ W1^T + b1)                 [B, T, D]
    alphas = softmax_t(u @ W2^T)                 [B, T, D]
    out    = sum_{b,t} h * alphas                [D]

Distribution: data-parallel over batch across 8 NeuronCores (4 batches per
core); small weights replicated; each core emits a partial [D] sum which the
host adds (the cross-core reduction is 2KB — not worth a collective).

Per-core dataflow (all in "transposed space", features on partitions):
    h^T   via DMA-xbar transpose (or PE identity-matmul) of bf16-cast h tiles
    u^T   = tanh(W1 @ h^T + b1)      TensorE (lhsT = W1^T blocks) + ScalarE
    s^T   = W2 @ u^T                 TensorE (lhsT = W2^T blocks)
    P     = exp(s^T)                 ScalarE, fused accum_out -> Z (denominator)
    N     = sum_t h^T * P            VectorE tensor_tensor_reduce
    out   = sum_b N_b / Z_b          tiny VectorE epilogue

exp needs no max-subtraction: |s| <= ||u||*||W2_row|| is bounded (~26 worst
case since |u|<1 via tanh), far below f32 overflow.
"""
import numpy as np
import ml_dtypes

import concourse.bacc as bacc
import concourse.bass as bass
import concourse.tile as tile
from concourse import bass_utils, mybir

F32 = mybir.dt.float32
BF16 = mybir.dt.bfloat16
Act = mybir.ActivationFunctionType
Alu = mybir.AluOpType

B, T, D = 32, 4096, 512
N_CORES = 8
B_LOC = B // N_CORES      # batches per core
TL = B_LOC * T            # rows per core
TCH = 512                 # t-chunk size
NCH = T // TCH            # chunks per batch
NT = D // 128             # 128-partition tiles per feature dim

TRANSPOSE = "pe"          # "xbar" | "xbar2" | "pe" | "pe2"


def build(repeat=1, transpose=TRANSPOSE, loop=False, cast="dma", ab="",
          split_psum=False, bufs=3, ct="tr2mm2"):
    # ct: col-tiling via tile_position — concurrent M=32/64 matmuls per
    #     128-wide tile so LDWEIGHTS of one col-group overlaps matmuls on
    #     other subarrays. "tr2mm2" (2x everywhere) measured best: 4x tiling
    #     pays too much PE-sequencer instruction overhead, no tiling pays
    #     serialized weight loads.
    # ab: timing-only ablations ("notr" drop transposes, "nomm2" drop mm2,
    #     "nonum" drop numerator mul+reduce). Results become WRONG; use only
    #     to attribute time on hardware.
    nc = bacc.Bacc("TRN2", target_bir_lowering=False, debug=False)
    h = nc.dram_tensor("h", [TL, D], F32, kind="ExternalInput")
    w1t = nc.dram_tensor("w1t", [D, D], BF16, kind="ExternalInput")  # W1.T
    w2t = nc.dram_tensor("w2t", [D, D], BF16, kind="ExternalInput")  # W2.T
    b1 = nc.dram_tensor("b1", [D, 1], F32, kind="ExternalInput")
    if transpose in ("pe", "pe2"):
        identt = nc.dram_tensor("ident", [128, 128], BF16,
                                kind="ExternalInput")
    out = nc.dram_tensor("out", [NT, 128], F32, kind="ExternalOutput")

    # row idx = chunk*TCH + s*128 + p  ->  view [chunk, p, s, d]
    hv = h.ap().rearrange("(ch s p) d -> ch p s d", s=4, p=128)

    with tile.TileContext(nc) as tc:
        import contextlib
        stk = contextlib.ExitStack()
        wp = stk.enter_context(tc.tile_pool(name="wts", bufs=1))
        hp = stk.enter_context(tc.tile_pool(name="hnat", bufs=bufs))
        hbp = stk.enter_context(tc.tile_pool(name="hbf", bufs=bufs))
        htp = stk.enter_context(tc.tile_pool(name="hT", bufs=bufs))
        up_ = stk.enter_context(tc.tile_pool(name="u", bufs=bufs))
        pp_ = stk.enter_context(tc.tile_pool(name="P", bufs=bufs))
        scp = stk.enter_context(tc.tile_pool(name="scr", bufs=bufs))
        znp = stk.enter_context(tc.tile_pool(name="zn", bufs=2))
        smp = stk.enter_context(tc.tile_pool(name="small", bufs=4))
        resp = stk.enter_context(tc.tile_pool(name="res", bufs=1))
        if split_psum:
            pstp = stk.enter_context(tc.tile_pool(name="pst", bufs=2, space="PSUM"))
            ps1p = stk.enter_context(tc.tile_pool(name="ps1", bufs=3, space="PSUM"))
            ps2p = stk.enter_context(tc.tile_pool(name="ps2", bufs=3, space="PSUM"))
        else:
            psp = stk.enter_context(tc.tile_pool(name="ps", bufs=8, space="PSUM"))
            pstp = ps1p = ps2p = psp
        shared = not split_psum
        tg = (lambda t: "ps") if shared else (lambda t: t)
        if True:

            w1_sb = wp.tile([128, NT, D], BF16)
            nc.sync.dma_start(
                w1_sb[:], w1t.ap().rearrange("(kk p) e -> p kk e", p=128))
            w2_sb = wp.tile([128, NT, D], BF16)
            nc.sync.dma_start(
                w2_sb[:], w2t.ap().rearrange("(kk p) e -> p kk e", p=128))
            b1_sb = wp.tile([128, NT, 1], F32)
            nc.sync.dma_start(
                b1_sb[:], b1.ap().rearrange("(i p) o -> p i o", p=128))
            if transpose in ("pe", "pe2"):
                ident = wp.tile([128, 128], BF16)
                nc.sync.dma_start(ident[:], identt.ap()[:])

            acc = resp.tile([128, NT], F32)
            nc.vector.memset(acc[:], 0.0)

            def batch_body(b):
                Zc = znp.tile([128, NT, NCH], F32, tag="Zc")
                Nc = znp.tile([128, NT, NCH], F32, tag="Nc")
                for c in range(NCH):
                    ci = b * NCH + c
                    if cast == "dma":
                        h_bf = hbp.tile([128, 4, TCH], BF16, tag="hbf")
                        nc.gpsimd.dma_start(h_bf[:], hv[ci])
                    else:
                        h_nat = hp.tile([128, 4, TCH], F32, tag="hnat")
                        nc.sync.dma_start(h_nat[:], hv[ci])
                        h_bf = hbp.tile([128, 4, TCH], BF16, tag="hbf")
                        nc.vector.tensor_copy(h_bf[:], h_nat[:])
                    hT = htp.tile([128, NT, TCH], BF16, tag="hT")
                    if ab in ("notr", "core"):
                        nc.vector.tensor_copy(hT[:], h_bf[:])
                    elif transpose in ("xbar", "xbar2"):
                        engs = ([nc.sync, nc.scalar] if transpose == "xbar2"
                                else [nc.sync])
                        for dd in range(NT):
                            for s in range(4):
                                engs[(dd * 4 + s) % len(engs)].dma_start(
                                    hT[:, dd, bass.ts(s, 128)],
                                    h_bf[:, s, bass.ts(dd, 128)],
                                    transpose=True)
                    elif transpose == "pe":
                        for dd in range(NT):
                            pst = pstp.tile([128, TCH], F32, tag=tg("pst"))
                            for s in range(4):
                                if ct in ("tr", "both", "trmm2"):
                                    for j in range(4):
                                        nc.tensor.matmul(
                                            pst[32 * j:32 * j + 32,
                                                bass.ts(s, 128)],
                                            h_bf[:, s,
                                                 bass.ds(dd * 128 + 32 * j, 32)],
                                            ident[:], start=True, stop=True,
                                            tile_position=(0, 32 * j))
                                elif ct in ("tr2", "tr2mm2"):
                                    for j in range(2):
                                        nc.tensor.matmul(
                                            pst[64 * j:64 * j + 64,
                                                bass.ts(s, 128)],
                                            h_bf[:, s,
                                                 bass.ds(dd * 128 + 64 * j, 64)],
                                            ident[:], start=True, stop=True,
                                            tile_position=(0, 64 * j))
                                else:
                                    nc.tensor.matmul(
                                        pst[:, bass.ts(s, 128)],
                                        h_bf[:, s, bass.ts(dd, 128)],
                                        ident[:], start=True, stop=True)
                            nc.vector.tensor_copy(hT[:, dd, :], pst[:])
                    else:  # pe2: transpose-mode matmul, bf16 PSUM output
                        for dd in range(NT):
                            pst = pstp.tile([128, TCH], BF16, tag=tg("pst"))
                            for s in range(4):
                                nc.tensor.transpose(
                                    pst[:, bass.ts(s, 128)],
                                    h_bf[:, s, bass.ts(dd, 128)],
                                    ident[:])
                            nc.vector.tensor_copy(hT[:, dd, :], pst[:])
                    # ---- mm1 + tanh ----
                    u_sb = up_.tile([128, NT, TCH], BF16, tag="u")
                    for mm in range(NT):
                        ps = ps1p.tile([128, TCH], F32, tag=tg("ps1"))
                        for kk in range(NT):
                            if ct in ("mm", "both"):
                                for j in range(4):
                                    nc.tensor.matmul(
                                        ps[32 * j:32 * j + 32, :],
                                        w1_sb[:, kk,
                                              bass.ds(mm * 128 + 32 * j, 32)],
                                        hT[:, kk, :],
                                        start=(kk == 0), stop=(kk == NT - 1),
                                        tile_position=(0, 32 * j),
                                        skip_group_check=True)
                            elif ct in ("trmm2", "tr2mm2"):
                                for j in range(2):
                                    nc.tensor.matmul(
                                        ps[64 * j:64 * j + 64, :],
                                        w1_sb[:, kk,
                                              bass.ds(mm * 128 + 64 * j, 64)],
                                        hT[:, kk, :],
                                        start=(kk == 0), stop=(kk == NT - 1),
                                        tile_position=(0, 64 * j),
                                        skip_group_check=True)
                            else:
                                nc.tensor.matmul(
                                    ps[:],
                                    w1_sb[:, kk, bass.ds(mm * 128, 128)],
                                    hT[:, kk, :],
                                    start=(kk == 0), stop=(kk == NT - 1))
                        nc.scalar.activation(
                            u_sb[:, mm, :], ps[:], Act.Tanh,
                            bias=b1_sb[:, mm, :])
                    # ---- mm2 + exp (Z via fused accumulate) ----
                    P_sb = pp_.tile([128, NT, TCH], BF16, tag="P")
                    for me in range(NT):
                        ps = ps2p.tile([128, TCH], F32, tag=tg("ps2"))
                        if ab != "nomm2":
                            for kk in range(NT):
                                if ct in ("mm", "both"):
                                    for j in range(4):
                                        nc.tensor.matmul(
                                            ps[32 * j:32 * j + 32, :],
                                            w2_sb[:, kk,
                                                  bass.ds(me * 128 + 32 * j, 32)],
                                            u_sb[:, kk, :],
                                            start=(kk == 0),
                                            stop=(kk == NT - 1),
                                            tile_position=(0, 32 * j),
                                            skip_group_check=True)
                                elif ct in ("trmm2", "tr2mm2"):
                                    for j in range(2):
                                        nc.tensor.matmul(
                                            ps[64 * j:64 * j + 64, :],
                                            w2_sb[:, kk,
                                                  bass.ds(me * 128 + 64 * j, 64)],
                                            u_sb[:, kk, :],
                                            start=(kk == 0),
                                            stop=(kk == NT - 1),
                                            tile_position=(0, 64 * j),
                                            skip_group_check=True)
                                else:
                                    nc.tensor.matmul(
                                        ps[:],
                                        w2_sb[:, kk, bass.ds(me * 128, 128)],
                                        u_sb[:, kk, :],
                                        start=(kk == 0), stop=(kk == NT - 1))
                        else:
                            nc.tensor.matmul(
                                ps[:], w2_sb[:, 0, bass.ds(me * 128, 128)],
                                u_sb[:, 0, :], start=True, stop=True)
                        nc.scalar.activation(
                            P_sb[:, me, :], ps[:], Act.Exp,
                            accum_out=Zc[:, me, c:c + 1])
                    # ---- numerator: Q = h^T * P, N_c = sum_t Q ----
                    if ab not in ("nonum", "core"):
                        q = scp.tile([128, NT, TCH], BF16, tag="sc")
                        nc.vector.tensor_mul(q[:], hT[:], P_sb[:])
                        nc.vector.tensor_reduce(
                            Nc[:, :, c:c + 1], q[:],
                            axis=mybir.AxisListType.X, op=Alu.add)
                    elif c == 0:
                        nc.vector.memset(Nc[:], 1.0)
                # ---- batch epilogue: acc += N/Z ----
                for me in range(NT):
                    zb = smp.tile([128, 1], F32, tag="zb")
                    nc.vector.tensor_reduce(
                        zb[:], Zc[:, me, :], axis=mybir.AxisListType.X,
                        op=Alu.add)
                    rz = smp.tile([128, 1], F32, tag="rz")
                    nc.vector.reciprocal(rz[:], zb[:])
                    nb = smp.tile([128, 1], F32, tag="nb")
                    nc.vector.tensor_reduce(
                        nb[:], Nc[:, me, :], axis=mybir.AxisListType.X,
                        op=Alu.add)
                    pr = smp.tile([128, 1], F32, tag="pr")
                    nc.vector.tensor_mul(pr[:], nb[:], rz[:])
                    nc.vector.tensor_add(
                        acc[:, me:me + 1], acc[:, me:me + 1], pr[:])

            if loop and repeat > 1:
                with tc.For_i(0, repeat, 1):
                    for b in range(B_LOC):
                        batch_body(b)
            else:
                for _rep in range(repeat):
                    for b in range(B_LOC):
                        batch_body(b)

            nc.sync.dma_start(out.ap().rearrange("i p -> p i"), acc[:])
            stk.close()

    nc.compile()
    return nc


def make_in_maps(hidden_states, W1, b1v, W2, transpose=TRANSPOSE):
    h = np.ascontiguousarray(np.asarray(hidden_states, dtype=np.float32))
    W1T = np.ascontiguousarray(np.asarray(W1, np.float32).T).astype(
        ml_dtypes.bfloat16)
    W2T = np.ascontiguousarray(np.asarray(W2, np.float32).T).astype(
        ml_dtypes.bfloat16)
    b1c = np.asarray(b1v, np.float32).reshape(D, 1).copy()
    hs = h.reshape(N_CORES, TL, D)
    maps = []
    for i in range(N_CORES):
        m = {"h": hs[i], "w1t": W1T, "w2t": W2T, "b1": b1c}
        if transpose in ("pe", "pe2"):
            m["ident"] = np.eye(128, dtype=ml_dtypes.bfloat16)
        maps.append(m)
    return maps


_NC_CACHE = {}


def _get_nc():
    if "nc" not in _NC_CACHE:
        _NC_CACHE["nc"] = build(repeat=1)
    return _NC_CACHE["nc"]


def kernel(hidden_states, W1, b1, W2):
    assert np.asarray(hidden_states).shape == (B, T, D)
    in_maps = make_in_maps(hidden_states, W1, b1, W2)
    nc = _get_nc()
    res = bass_utils.run_bass_kernel_spmd(
        nc, in_maps, core_ids=list(range(N_CORES)), trace=False)
    total = np.zeros(D, np.float64)
    for r in res.results:
        total += r["out"].reshape(D).astype(np.float64)
    return total.astype(np.float32)



# revision 12
# speedup vs baseline: 280.8841x; 280.8841x over previous
"""Trainium2 Bass kernel for nn_Attention_28948079575569 (softmax pooling).

Computation (reference):
    u      = tanh(h @ W1^T + b1)                 [B, T, D]
    alphas = softmax_t(u @ W2^T)                 [B, T, D]
    out    = sum_{b,t} h * alphas                [D]

Distribution: data-parallel over batch across 8 NeuronCores (4 batches per
core); small weights replicated; each core emits a partial [D] sum which the
host adds (the cross-core reduction is 2KB — not worth a collective).

Per-core dataflow (transposed space, features on partitions). h is host-cast
to bf16 (numerically identical to the previous on-chip DMA cast) so the
DMA xbar can transpose it HBM->SBUF directly:
    h^T   via dma_start_transpose from HBM       [128, NT, TCH] bf16
    u^T   = tanh(W1 @ h^T + b1)      TensorE (lhsT = W1^T blocks) + ScalarE
    s^T   = W2 @ u^T                 TensorE (lhsT = W2^T blocks)
    P     = exp(s^T)                 ScalarE, fused accum_out -> Z (denominator)
    N     = sum_t h^T * P            VectorE fused tensor_tensor_reduce
    out   = sum_b N_b / Z_b          small VectorE epilogue per batch

exp needs no max-subtraction: |s| <= ||u||*||W2_row|| is bounded (~26 worst
case since |u|<1 via tanh), far below f32 overflow.
"""
import numpy as np
import ml_dtypes

import concourse.bacc as bacc
import concourse.bass as bass
import concourse.tile as tile
from concourse import bass_utils, mybir

F32 = mybir.dt.float32
BF16 = mybir.dt.bfloat16
Act = mybir.ActivationFunctionType
Alu = mybir.AluOpType

B, T, D = 32, 4096, 512
N_CORES = 8
B_LOC = B // N_CORES      # batches per core
TL = B_LOC * T            # rows per core
TCH = 512                 # t-chunk size
NCH = T // TCH            # chunks per batch
NT = D // 128             # 128-partition tiles per feature dim


def build(repeat=1, loop=False, ab="", bufs=4, ct=2, psum_bufs=8,
          xb_eng="alt", fuse_num=False, batched_epi=True):
    # fuse_num=True (vector.tensor_tensor_reduce) hangs the device on this
    # runtime — keep the separate mul+reduce; the numerator is fully hidden
    # under TensorE anyway (measured 0 marginal cost).
    # ct: col-tiling factor via tile_position (2 = two concurrent M=64
    #     matmuls so LDWEIGHTS of one col-group overlaps the other; measured
    #     best in the earlier PE-transpose design).
    # ab: timing-only ablations ("nonum" drop numerator, "nomm2" shrink mm2,
    #     "notr" replace the transposed load with a plain load of the same
    #     bytes). Results become WRONG; only for time attribution.
    nc = bacc.Bacc("TRN2", target_bir_lowering=False, debug=False)
    h = nc.dram_tensor("h", [TL, D], BF16, kind="ExternalInput")
    w1t = nc.dram_tensor("w1t", [D, D], BF16, kind="ExternalInput")  # W1.T
    w2t = nc.dram_tensor("w2t", [D, D], BF16, kind="ExternalInput")  # W2.T
    b1 = nc.dram_tensor("b1", [D, 1], F32, kind="ExternalInput")
    out = nc.dram_tensor("out", [NT, 128], F32, kind="ExternalOutput")

    hv = h.ap().rearrange("(ch t) d -> ch t d", t=TCH)  # [32, 512, 512]
    # plain-load view for the "notr" ablation (same bytes, no xbar)
    hv_pl = h.ap().rearrange("(ch s p) d -> ch p s d", s=4, p=128)

    with tile.TileContext(nc) as tc:
        import contextlib
        stk = contextlib.ExitStack()
        wp = stk.enter_context(tc.tile_pool(name="wts", bufs=1))
        htp = stk.enter_context(tc.tile_pool(name="hT", bufs=bufs))
        up_ = stk.enter_context(tc.tile_pool(name="u", bufs=bufs))
        pp_ = stk.enter_context(tc.tile_pool(name="P", bufs=bufs))
        scp = stk.enter_context(tc.tile_pool(name="scr", bufs=2))
        znp = stk.enter_context(tc.tile_pool(name="zn", bufs=2))
        smp = stk.enter_context(tc.tile_pool(name="small", bufs=4))
        resp = stk.enter_context(tc.tile_pool(name="res", bufs=1))
        psp = stk.enter_context(
            tc.tile_pool(name="ps", bufs=psum_bufs, space="PSUM"))
        if True:
            w1_sb = wp.tile([128, NT, D], BF16)
            nc.sync.dma_start(
                w1_sb[:], w1t.ap().rearrange("(kk p) e -> p kk e", p=128))
            w2_sb = wp.tile([128, NT, D], BF16)
            nc.sync.dma_start(
                w2_sb[:], w2t.ap().rearrange("(kk p) e -> p kk e", p=128))
            b1_sb = wp.tile([128, NT, 1], F32)
            nc.sync.dma_start(
                b1_sb[:], b1.ap().rearrange("(i p) o -> p i o", p=128))

            acc = resp.tile([128, NT], F32)
            nc.vector.memset(acc[:], 0.0)

            def mm(ps, w_sb, rhs, me, kk):
                if ct == 2:
                    for j in range(2):
                        nc.tensor.matmul(
                            ps[64 * j:64 * j + 64, :],
                            w_sb[:, kk, bass.ds(me * 128 + 64 * j, 64)],
                            rhs,
                            start=(kk == 0), stop=(kk == NT - 1),
                            tile_position=(0, 64 * j),
                            skip_group_check=True)
                else:
                    nc.tensor.matmul(
                        ps[:], w_sb[:, kk, bass.ds(me * 128, 128)], rhs,
                        start=(kk == 0), stop=(kk == NT - 1))

            def stage1(ci):
                c = ci % NCH
                hT = htp.tile([128, NT, TCH], BF16, tag="hT")
                if ab == "notr":
                    nc.sync.dma_start(hT[:], hv_pl[ci])
                else:
                    eng = (nc.sync if xb_eng == "sync"
                           else [nc.sync, nc.scalar][c % 2])
                    eng.dma_start_transpose(hT[:], hv[ci])
                # ---- mm1 + tanh ----
                u_sb = up_.tile([128, NT, TCH], BF16, tag="u")
                for me in range(NT):
                    ps = psp.tile([128, TCH], F32, tag="ps")
                    for kk in range(NT):
                        mm(ps, w1_sb, hT[:, kk, :], me, kk)
                    nc.scalar.activation(
                        u_sb[:, me, :], ps[:], Act.Tanh,
                        bias=b1_sb[:, me, :])
                return hT, u_sb

            def stage2(ci, hT, u_sb, Zc, Nc):
                c = ci % NCH
                # ---- mm2 + exp (Z via fused accumulate) ----
                P_sb = pp_.tile([128, NT, TCH], BF16, tag="P")
                for me in range(NT):
                    ps = psp.tile([128, TCH], F32, tag="ps")
                    if ab == "nomm2":
                        nc.tensor.matmul(
                            ps[:], w2_sb[:, 0, bass.ds(me * 128, 128)],
                            u_sb[:, 0, :], start=True, stop=True)
                    else:
                        for kk in range(NT):
                            mm(ps, w2_sb, u_sb[:, kk, :], me, kk)
                    nc.scalar.activation(
                        P_sb[:, me, :], ps[:], Act.Exp,
                        accum_out=Zc[:, me, c:c + 1])
                # ---- numerator: Nc[:,me,c] = sum_t h^T * P ----
                if ab != "nonum":
                    if fuse_num:
                        for me in range(NT):
                            q = scp.tile([128, TCH], BF16, tag="sc")
                            nc.vector.tensor_tensor_reduce(
                                q[:], hT[:, me, :], P_sb[:, me, :],
                                scale=1.0, scalar=0.0,
                                op0=Alu.mult, op1=Alu.add,
                                accum_out=Nc[:, me, c:c + 1])
                    else:
                        q = scp.tile([128, NT, TCH], BF16, tag="sc")
                        nc.vector.tensor_mul(q[:], hT[:], P_sb[:])
                        nc.vector.tensor_reduce(
                            Nc[:, :, c:c + 1], q[:],
                            axis=mybir.AxisListType.X, op=Alu.add)
                elif c == 0:
                    nc.vector.memset(Nc[:], 1.0)

            def epilogue(Zc, Nc):
                # ---- batch epilogue: acc += N/Z (batched per-me ops) ----
                if batched_epi:
                    zb = smp.tile([128, NT], F32, tag="zb")
                    nc.vector.tensor_reduce(
                        zb[:], Zc[:], axis=mybir.AxisListType.X, op=Alu.add)
                    rz = smp.tile([128, NT], F32, tag="rz")
                    nc.vector.reciprocal(rz[:], zb[:])
                    nb = smp.tile([128, NT], F32, tag="nb")
                    nc.vector.tensor_reduce(
                        nb[:], Nc[:], axis=mybir.AxisListType.X, op=Alu.add)
                    pr = smp.tile([128, NT], F32, tag="pr")
                    nc.vector.tensor_mul(pr[:], nb[:], rz[:])
                    nc.vector.tensor_add(acc[:], acc[:], pr[:])
                else:
                    for me in range(NT):
                        zb = smp.tile([128, 1], F32, tag="zb")
                        nc.vector.tensor_reduce(
                            zb[:], Zc[:, me, :], axis=mybir.AxisListType.X,
                            op=Alu.add)
                        rz = smp.tile([128, 1], F32, tag="rz")
                        nc.vector.reciprocal(rz[:], zb[:])
                        nb = smp.tile([128, 1], F32, tag="nb")
                        nc.vector.tensor_reduce(
                            nb[:], Nc[:, me, :], axis=mybir.AxisListType.X,
                            op=Alu.add)
                        pr = smp.tile([128, 1], F32, tag="pr")
                        nc.vector.tensor_mul(pr[:], nb[:], rz[:])
                        nc.vector.tensor_add(
                            acc[:, me:me + 1], acc[:, me:me + 1], pr[:])

            def repeat_body():
                # software pipeline: stage2(ci-1) is emitted after
                # stage1(ci), so mm2 never waits on the tanh of its own
                # chunk; drains at the end of each repeat iteration.
                zn = {}
                pending = None
                for ci in range(B_LOC * NCH):
                    b, c = divmod(ci, NCH)
                    if c == 0:
                        Zt = znp.tile([128, NT, NCH], F32, tag="Zc")
                        Nt = znp.tile([128, NT, NCH], F32, tag="Nc")
                        zn[b] = (Zt, Nt)
                    hT, u_sb = stage1(ci)
                    if pending is not None:
                        pci, phT, pu = pending
                        pb = pci // NCH
                        stage2(pci, phT, pu, *zn[pb])
                        if pci % NCH == NCH - 1:
                            epilogue(*zn.pop(pb))
                    pending = (ci, hT, u_sb)
                pci, phT, pu = pending
                pb = pci // NCH
                stage2(pci, phT, pu, *zn[pb])
                epilogue(*zn.pop(pb))

            if loop and repeat > 1:
                with tc.For_i(0, repeat, 1):
                    repeat_body()
            else:
                for _rep in range(repeat):
                    repeat_body()

            nc.sync.dma_start(out.ap().rearrange("i p -> p i"), acc[:])
            stk.close()

    nc.compile()
    return nc


def make_in_maps(hidden_states, W1, b1v, W2):
    h = np.asarray(hidden_states, dtype=np.float32).astype(ml_dtypes.bfloat16)
    W1T = np.ascontiguousarray(np.asarray(W1, np.float32).T).astype(
        ml_dtypes.bfloat16)
    W2T = np.ascontiguousarray(np.asarray(W2, np.float32).T).astype(
        ml_dtypes.bfloat16)
    b1c = np.asarray(b1v, np.float32).reshape(D, 1).copy()
    hs = np.ascontiguousarray(h.reshape(N_CORES, TL, D))
    return [{"h": hs[i], "w1t": W1T, "w2t": W2T, "b1": b1c}
            for i in range(N_CORES)]


_NC_CACHE = {}


def _get_nc():
    if "nc" not in _NC_CACHE:
        _NC_CACHE["nc"] = build(repeat=1)
    return _NC_CACHE["nc"]


def kernel(hidden_states, W1, b1, W2):
    assert np.asarray(hidden_states).shape == (B, T, D)
    in_maps = make_in_maps(hidden_states, W1, b1, W2)
    nc = _get_nc()
    res = bass_utils.run_bass_kernel_spmd(
        nc, in_maps, core_ids=list(range(N_CORES)), trace=False)
    total = np.zeros(D, np.float64)
    for r in res.results:
        total += r["out"].reshape(D).astype(np.float64)
    return total.astype(np.float32)
